# revision 10
# baseline (speedup 1.0000x reference)
"""Trainium2 Bass kernel for nn_ConeIntersection.

Strategy: pure data-parallel over B (8 cores x 1024 tokens). Host pre-tiles
inputs to [NBT, HEADS, N, 2, 128, TB] so every DMA is one fully-contiguous
1MB read; concat([axis - arg/2, axis + arg/2]) folded into effective weights;
mean-over-N of the gate folded into PSUM accumulation; sigmoid folded into
tanh (same ACT table as exp/relu) so each token-tile needs only two ACT
table sets; softmax ratio cancels the denominator so only a clamped divide +
full-range arctan + bitwise quadrant fix is needed; sin/cos/exp products and
over-N sums run in fp16 (2x DVE rate, fp16 mantissa needed: bf16 cancels catastrophically); min-over-N and the gate multiply run
on the otherwise idle GpSimd engine.
"""
import sys
sys.path.insert(0, '/opt/trn_rl_repo')
import numpy as np
from contextlib import ExitStack

N, B, DIM, HEADS = 4, 8192, 1024, 4
HD = DIM // HEADS            # 256
NCORES = 8
BL = B // NCORES             # 1024 tokens per core
TB = 256                     # token tile (matmul free dim)
NBT = BL // TB               # 4 mega-tiles
PI = 3.141592653589793
HALF_PI = PI / 2

_CACHE = {}


def _build():
    from concourse import bacc, tile, mybir
    f32 = mybir.dt.float32
    f32r = mybir.dt.float32r
    f16 = mybir.dt.float16
    i32 = mybir.dt.int32
    AF = mybir.ActivationFunctionType
    ALU = mybir.AluOpType

    nc = bacc.Bacc("TRN2", target_bir_lowering=False, debug=False,
                   num_devices=NCORES)

    axisT_d = nc.dram_tensor("axisT", [NBT, HEADS, N, 2, 128, TB], f32,
                             kind="ExternalInput")
    argT_d = nc.dram_tensor("argT", [NBT, HEADS, N, 2, 128, TB], f32,
                            kind="ExternalInput")
    wds = {}
    for wname in ["waax", "waar", "wgax", "wgar", "w2a", "w2g"]:
        wds[wname] = nc.dram_tensor(wname, [2 * 128, HD], f32, kind="ExternalInput")
    bds = {}
    for bname in ["b1a", "b1g", "b2a", "b2gh"]:
        bds[bname] = nc.dram_tensor(bname, [2, 128], f32, kind="ExternalInput")
    axo_d = nc.dram_tensor("axis_outT", [NBT, HEADS, 2, 128, TB], f32,
                           kind="ExternalOutput")
    ago_d = nc.dram_tensor("arg_outT", [NBT, HEADS, 2, 128, TB], f32,
                           kind="ExternalOutput")

    with tile.TileContext(nc) as tc, ExitStack() as ctx:
        wpool = ctx.enter_context(tc.tile_pool(name="w", bufs=1))
        atp = ctx.enter_context(tc.tile_pool(name="atp", bufs=4))
        gtp = ctx.enter_context(tc.tile_pool(name="gtp", bufs=2))
        h1p = ctx.enter_context(tc.tile_pool(name="h1p", bufs=4))
        expp = ctx.enter_context(tc.tile_pool(name="expp", bufs=4))
        tmpp = ctx.enter_context(tc.tile_pool(name="tmpp", bufs=2))
        prdp = ctx.enter_context(tc.tile_pool(name="prdp", bufs=5))
        sump = ctx.enter_context(tc.tile_pool(name="sump", bufs=8))
        rcp = ctx.enter_context(tc.tile_pool(name="rcp", bufs=4))
        gatep = ctx.enter_context(tc.tile_pool(name="gatep", bufs=2))
        perp = ctx.enter_context(tc.tile_pool(name="perp", bufs=5))
        outp = ctx.enter_context(tc.tile_pool(name="outp", bufs=2))
        pmm = ctx.enter_context(tc.tile_pool(name="pmm", bufs=2, space="PSUM"))
        psc = ctx.enter_context(tc.tile_pool(name="psc", bufs=1, space="PSUM"))
        pgt = ctx.enter_context(tc.tile_pool(name="pgt", bufs=2, space="PSUM"))

        # resident weights / biases
        w_sb = {}
        for wname, wd in wds.items():
            tls = []
            for i in range(2):
                t = wpool.tile([128, HD], f32, tag=f"w_{wname}_{i}")
                nc.sync.dma_start(t[:].bitcast(f32r), wd[i * 128:(i + 1) * 128, :].bitcast(f32r))
                tls.append(t)
            w_sb[wname] = tls
        b_sb = {}
        for bname, bd in bds.items():
            tls = []
            for j in range(2):
                t = wpool.tile([128, 1], f32, tag=f"b_{bname}_{j}")
                nc.sync.dma_start(t[:], bd[j].unsqueeze(1))
                tls.append(t)
            b_sb[bname] = tls

        a0_last = None     # last arctan output of previous bt (ACT-phase gate)
        for bt in range(NBT):
            at_h = {}
            exp_h = {}
            minv = {}
            tg_h = {}
            # ---------------- PHASE A (table: exp_and_others) -------------
            # gate tiles force this bt's A-activations to issue after the
            # previous bt's B2 arctans (ACT-table batching; value is exact).
            if a0_last is not None:
                gA1 = gatep.tile([128, 1], f32, tag="gA1")
                nc.vector.tensor_scalar(gA1[:], a0_last[:, 0, 0:1], 0.0, 1.0,
                                        ALU.mult, ALU.add)
                gA5 = gatep.tile([128, 1], f32, tag="gA5")
                nc.vector.tensor_scalar(gA5[:], a0_last[:, 0, 0:1], 0.0, 0.5,
                                        ALU.mult, ALU.add)
            else:
                gA1 = gatep.tile([128, 1], f32, tag="gA1")
                nc.vector.memset(gA1[:], 1.0)
                gA5 = gatep.tile([128, 1], f32, tag="gA5")
                nc.vector.memset(gA5[:], 0.5)
            for h in range(HEADS):
                at = atp.tile([128, N, 2, TB], f32, tag="at")
                nc.sync.dma_start(at[:].bitcast(f32r),
                                  axisT_d[bt, h].rearrange("n j p t -> p n j t").bitcast(f32r))
                gt = gtp.tile([128, N, 2, TB], f32, tag="gt")
                nc.sync.dma_start(gt[:].bitcast(f32r),
                                  argT_d[bt, h].rearrange("n j p t -> p n j t").bitcast(f32r))
                at_h[h] = at

                # min over n on GpSimd (idle engine; overlaps matmuls)
                mv = perp.tile([128, 2, TB], f32, tag="minv")
                nc.vector.tensor_tensor(mv[:], gt[:, 0], gt[:, 1], ALU.min)
                nc.vector.tensor_tensor(mv[:], mv[:], gt[:, 2], ALU.min)
                nc.vector.tensor_tensor(mv[:], mv[:], gt[:, 3], ALU.min)
                minv[h] = mv

                # L1: h1a / h1g  [128(out j), N, TB]
                h1a, h1g = [], []
                for (wax, war, bias, hl) in (("waax", "waar", "b1a", h1a),
                                             ("wgax", "wgar", "b1g", h1g)):
                    for j in range(2):
                        pa = pmm.tile([128, N, TB], f32, tag="pmm")
                        wseq = [(w_sb[wax][0], 0, "a"), (w_sb[wax][1], 1, "a"),
                                (w_sb[war][0], 0, "g"), (w_sb[war][1], 1, "g")]
                        for half in ((0, 2), (1, 3)):
                            for wb, (wt, i, which) in enumerate(wseq):
                                for n in half:
                                    rhs = at[:, n, i, :] if which == "a" else gt[:, n, i, :]
                                    nc.tensor.matmul(pa[:, n, :],
                                                     wt[:, j * 128:(j + 1) * 128].bitcast(f32r),
                                                     rhs.bitcast(f32r),
                                                     start=(wb == 0), stop=(wb == 3))
                        ht = h1p.tile([128, N, TB], f32, tag="h1")
                        nc.scalar.activation(ht[:].bitcast(f32r), pa[:], AF.Relu,
                                             bias=b_sb[bias][j][:], scale=gA1[:])
                        hl.append(ht)

                # L2 scores -> exp (bf16); gate -> tanh trick
                ex = expp.tile([128, N, 2, TB], f16, tag="exp")
                for j in range(2):
                    ps = psc.tile([128, N, TB], f32, tag="psc")
                    for half in ((0, 2), (1, 3)):
                        for k, i in enumerate((0, 1)):
                            for n in half:
                                nc.tensor.matmul(ps[:, n, :],
                                                 w_sb["w2a"][i][:, j * 128:(j + 1) * 128].bitcast(f32r),
                                                 h1a[i][:, n, :].bitcast(f32r),
                                                 start=(k == 0), stop=(k == 1))
                    nc.scalar.activation(ex[:, :, j, :], ps[:], AF.Exp,
                                         bias=b_sb["b2a"][j][:], scale=gA1[:])
                exp_h[h] = ex

                # gate: sigmoid(z) = 0.5*tanh(z/2) + 0.5 ; tanh is in the exp table
                tg = perp.tile([128, 2, TB], f32, tag="tg")
                for j in range(2):
                    pg = pgt.tile([128, TB], f32, tag="pgt")
                    k = 0
                    for i in range(2):
                        for n in range(N):
                            nc.tensor.matmul(pg[:],
                                             w_sb["w2g"][i][:, j * 128:(j + 1) * 128].bitcast(f32r),
                                             h1g[i][:, n, :].bitcast(f32r),
                                             start=(k == 0), stop=(k == 2 * N - 1))
                            k += 1
                    nc.scalar.activation(tg[:, j, :], pg[:], AF.Tanh, scale=gA5[:],
                                         bias=b_sb["b2gh"][j][:])
                tg_h[h] = tg

            # gate outputs (no ACT needed beyond tanh above)
            for h in range(HEADS):
                t05 = outp.tile([128, 2, TB], f32, tag="t05")
                nc.vector.tensor_scalar(t05[:], tg_h[h][:], 0.5, 0.5,
                                        ALU.mult, ALU.add)
                nc.gpsimd.tensor_tensor(t05[:].rearrange("p j t -> p (j t)"),
                                        t05[:].rearrange("p j t -> p (j t)"),
                                        minv[h][:].rearrange("p j t -> p (j t)"),
                                        ALU.mult)
                nc.sync.dma_start(
                    ago_d[bt, h].rearrange("j p t -> p j t"), t05[:])

            # ---------------- PHASE B1 (table: trig_and_small; sin only) ----
            gS = gatep.tile([128, 1], f32, tag="gS")
            nc.vector.tensor_scalar(gS[:], tg_h[3][:, 0, 0:1], 0.0, None, ALU.mult)
            r_h = {}
            corr_h = {}
            last_cos = None
            for h in range(HEADS):
                at = at_h[h]
                ex = exp_h[h]
                fl = lambda t: t[:].rearrange("p a b t -> p (a b t)")
                xw = tmpp.tile([128, N, 2, TB], f32, tag="xw")
                nc.vector.add_range_wrap(fl(xw), fl(at), 0.0, PI, 2 * PI)
                sinv = prdp.tile([128, N, 2, TB], f16, tag="prd")
                nc.scalar.activation(fl(sinv), fl(xw), AF.Sin, bias=gS[:])
                xw2 = tmpp.tile([128, N, 2, TB], f32, tag="xw")
                nc.vector.add_range_wrap(fl(xw2), fl(at), HALF_PI, PI, 2 * PI)
                cosv = prdp.tile([128, N, 2, TB], f16, tag="prd")
                nc.scalar.activation(fl(cosv), fl(xw2), AF.Sin, bias=gS[:])
                last_cos = cosv

                ec = prdp.tile([128, N, 2, TB], f16, tag="prd")
                nc.vector.tensor_tensor(fl(ec), fl(ex), fl(cosv), ALU.mult)
                es = prdp.tile([128, N, 2, TB], f16, tag="prd")
                nc.vector.tensor_tensor(fl(es), fl(ex), fl(sinv), ALU.mult)

                # pairwise trees over n (fp16 2x, final level f32)
                c2 = sump.tile([128, 2, 2, TB], f16, tag="s")
                nc.vector.tensor_tensor(c2[:], ec[:, 0:2], ec[:, 2:4], ALU.add)
                s2 = sump.tile([128, 2, 2, TB], f16, tag="s")
                nc.vector.tensor_tensor(s2[:], es[:, 0:2], es[:, 2:4], ALU.add)
                e2 = sump.tile([128, 2, 2, TB], f16, tag="s")
                nc.vector.tensor_tensor(e2[:], ex[:, 0:2], ex[:, 2:4], ALU.add)
                sc = sump.tile([128, 2, TB], f32, tag="s")
                nc.vector.tensor_tensor(sc[:], c2[:, 0], c2[:, 1], ALU.add)
                ss = sump.tile([128, 2, TB], f32, tag="s")
                nc.vector.tensor_tensor(ss[:], s2[:, 0], s2[:, 1], ALU.add)
                se = sump.tile([128, 2, TB], f32, tag="s")
                nc.vector.tensor_tensor(se[:], e2[:, 0], e2[:, 1], ALU.add)

                # clamp: sc' = where(|sc| < 0.001*se, 0.001*se, sc)
                th = sump.tile([128, 2, TB], f32, tag="s")
                nc.vector.tensor_scalar(th[:], se[:], 0.001, None, ALU.mult)
                absc = sump.tile([128, 2, TB], f32, tag="s")
                nc.vector.tensor_scalar(absc[:].bitcast(i32), sc[:].bitcast(i32),
                                        0x7FFFFFFF, None, ALU.bitwise_and)
                mask = sump.tile([128, 2, TB], i32, tag="s")
                nc.vector.tensor_tensor(mask[:], absc[:], th[:], ALU.is_lt)
                nc.vector.copy_predicated(sc[:], mask[:], th[:])
                zr = sump.tile([128, 2, TB], f32, tag="s")
                nc.vector.reciprocal_approx_fast(zr[:], sc[:])
                r = rcp.tile([128, 2, TB], f32, tag="r")
                nc.gpsimd.tensor_tensor(r[:].rearrange("p j t -> p (j t)"),
                                        ss[:].rearrange("p j t -> p (j t)"),
                                        zr[:].rearrange("p j t -> p (j t)"), ALU.mult)
                # quadrant: corr = pi*(sc'<0) | signbit(ss)
                mpi = sump.tile([128, 2, TB], f32, tag="s")
                nc.vector.tensor_scalar(mpi[:], sc[:], 0.0, PI, ALU.is_lt, ALU.mult)
                sgn = sump.tile([128, 2, TB], i32, tag="s")
                nc.vector.tensor_scalar(sgn[:], ss[:].bitcast(i32), 0x80000000,
                                        None, ALU.bitwise_and)
                corr = rcp.tile([128, 2, TB], f32, tag="corr")
                nc.vector.tensor_tensor(corr[:].bitcast(i32), mpi[:].bitcast(i32),
                                        sgn[:], ALU.bitwise_or)
                r_h[h] = r
                corr_h[h] = corr

            # ---------------- PHASE B2 (table: sigmoid_and_others; arctan) --
            gB = gatep.tile([128, 1], f32, tag="gB")
            nc.vector.tensor_scalar(gB[:], last_cos[:, 0, 0, 0:1], 0.0, None,
                                    ALU.mult)
            for h in range(HEADS):
                a0 = outp.tile([128, 2, TB], f32, tag="a0")
                nc.scalar.activation(a0[:], r_h[h][:], AF.Arctan, bias=gB[:])
                if h == HEADS - 1:
                    a0_last = a0
                nc.gpsimd.tensor_tensor(a0[:].rearrange("p j t -> p (j t)"),
                                        a0[:].rearrange("p j t -> p (j t)"),
                                        corr_h[h][:].rearrange("p j t -> p (j t)"),
                                        ALU.add)
                nc.sync.dma_start(
                    axo_d[bt, h].rearrange("j p t -> p j t"), a0[:])

    nc.compile()
    return nc


def _get_nc():
    if "nc" not in _CACHE:
        _CACHE["nc"] = _build()
    return _CACHE["nc"]


def _prep_inputs(axis_embeddings, arg_embeddings, W_axis1, b_axis1, W_arg1,
                 b_arg1, W_axis2, b_axis2, W_arg2, b_arg2):
    f = np.float32
    W_axis1 = np.asarray(W_axis1, f); W_arg1 = np.asarray(W_arg1, f)
    W_axis2 = np.asarray(W_axis2, f); W_arg2 = np.asarray(W_arg2, f)
    # logits = [axis - arg/2, axis + arg/2]; fold concat into effective weights
    waax = np.ascontiguousarray((W_axis1[:, :HD] + W_axis1[:, HD:]).T)
    waar = np.ascontiguousarray(((W_axis1[:, HD:] - W_axis1[:, :HD]) / 2).T)
    wgax = np.ascontiguousarray((W_arg1[:, :HD] + W_arg1[:, HD:]).T)
    wgar = np.ascontiguousarray(((W_arg1[:, HD:] - W_arg1[:, :HD]) / 2).T)
    w2a = np.ascontiguousarray(W_axis2.T)
    w2g = np.ascontiguousarray((W_arg2 / N).T)     # folds mean over N
    weights = {"waax": waax, "waar": waar, "wgax": wgax, "wgar": wgar,
               "w2a": w2a, "w2g": w2g,
               "b1a": np.asarray(b_axis1, f).reshape(2, 128),
               "b1g": np.asarray(b_arg1, f).reshape(2, 128),
               "b2a": np.asarray(b_axis2, f).reshape(2, 128),
               "b2gh": np.asarray(b_arg2, f).reshape(2, 128) / 2}

    axis_embeddings = np.asarray(axis_embeddings, f)
    arg_embeddings = np.asarray(arg_embeddings, f)

    def tileize(x_l):
        # [N, BL, DIM] -> [NBT, HEADS, N, 2, 128, TB] contiguous
        v = x_l.reshape(N, NBT, TB, HEADS, 2, 128)
        return np.ascontiguousarray(v.transpose(1, 3, 0, 4, 5, 2))

    in_maps = []
    for c in range(NCORES):
        sl = slice(c * BL, (c + 1) * BL)
        m = dict(weights)
        m["axisT"] = tileize(axis_embeddings[:, sl, :])
        m["argT"] = tileize(arg_embeddings[:, sl, :])
        in_maps.append(m)
    return in_maps


def kernel(axis_embeddings, arg_embeddings, W_axis1, b_axis1, W_arg1, b_arg1,
           W_axis2, b_axis2, W_arg2, b_arg2, _return_results=False):
    from concourse.bass_utils import run_bass_kernel_spmd
    nc = _get_nc()
    in_maps = _prep_inputs(axis_embeddings, arg_embeddings, W_axis1, b_axis1,
                           W_arg1, b_arg1, W_axis2, b_axis2, W_arg2, b_arg2)
    res = run_bass_kernel_spmd(nc, in_maps, list(range(NCORES)))
    f = np.float32
    axis_out = np.empty((B, DIM), f)
    arg_out = np.empty((B, DIM), f)
    for c in range(NCORES):
        sl = slice(c * BL, (c + 1) * BL)
        # [NBT, HEADS, 2, 128, TB] -> [BL, DIM]
        ax = res.results[c]["axis_outT"].transpose(0, 4, 1, 2, 3).reshape(BL, DIM)
        ag = res.results[c]["arg_outT"].transpose(0, 4, 1, 2, 3).reshape(BL, DIM)
        axis_out[sl] = ax
        arg_out[sl] = ag
    if _return_results:
        return (axis_out, arg_out), res
    return axis_out, arg_out
